# revision 3
# baseline (speedup 1.0000x reference)
"""Dinov3 self-attention Bass kernel for TRN2 — v2 (pipelined).

Sharding: data-parallel over batch. B=8 -> 8 NeuronCores, weights replicated.

Key structure vs v1:
  - x/Wq/sin/cos transposed on the (idle) PE via identity matmuls; Wk/Wv/Wp
    via async DMA-transpose on scalar/gpsimd queues. Loads spread over queues.
  - attention: per head pair, QK scores go to 3-bank psum tiles [128, 1374]
    so exp is ONE ScalarE instruction per (head, j-tile) (132 total).
  - QK packs the two heads into PE row groups (0,0)/(64,0) -> concurrent.
  - PV accumulates j-windows (2 j-tiles) in 1-bank psum, then DVE adds into
    an SBUF accumulator [65, S]; row 64 = ones-fold softmax denominator.
  - normalize: gpsimd partition_broadcast + DVE reciprocal_approx_fast
    (no DRAM bounces).
  - projections for later pairs and v-proj are emitted as "filler" PE work
    inside the ACT-bound attention loop, with explicit deadline checkpoints.
"""

import contextlib
import sys

import numpy as np

sys.path.insert(0, "/opt/trn_rl_repo")

import concourse.bacc as bacc
import concourse.bass as bass
import concourse.tile as tile
from concourse import mybir

S = 1374
H = 768
NH = 12
D = 64
NROT = 1369
PREFIX = S - NROT  # 5
B = 8

P = 128
NKT = H // P  # 6 o/kt tiles
NSTILE = (S + P - 1) // P  # 11 s-tiles, last has 94 rows
BANK = 512

ICH3 = ((0, 512), (512, 512), (1024, S - 1024))  # i-chunks in 3-bank tile
VCH = ((0, 512), (512, 256))  # v/out-proj o-chunks
JWIN = [(0, 1, 2), (3, 4, 5), (6, 7, 8), (9, 10)]
NPAIR = NH // 2  # 6

F32 = mybir.dt.float32
BF16 = mybir.dt.bfloat16
EXP = mybir.ActivationFunctionType.Exp
ALU = mybir.AluOpType


def _stile(i):
    s0 = i * P
    return s0, min(P, S - s0)


def build_kernel(nc):
    x_ext = nc.declare_dram_parameter("hidden_states", [S, H], F32, isOutput=False)
    sin_ext = nc.declare_dram_parameter("sin", [NROT, D], F32, isOutput=False)
    cos_ext = nc.declare_dram_parameter("cos", [NROT, D], F32, isOutput=False)
    wq_ext = nc.declare_dram_parameter("Wq", [H, H], F32, isOutput=False)
    bq_ext = nc.declare_dram_parameter("bq", [H], F32, isOutput=False)
    wk_ext = nc.declare_dram_parameter("Wk", [H, H], F32, isOutput=False)
    wv_ext = nc.declare_dram_parameter("Wv", [H, H], F32, isOutput=False)
    bv_ext = nc.declare_dram_parameter("bv", [H], F32, isOutput=False)
    wp_ext = nc.declare_dram_parameter("Wp", [H, H], F32, isOutput=False)
    bp_ext = nc.declare_dram_parameter("bp", [H], F32, isOutput=False)
    out_ext = nc.declare_dram_parameter("out", [S, H], F32, isOutput=True)

    with tile.TileContext(nc) as tc:
        _body(tc, x_ext, sin_ext, cos_ext, wq_ext, bq_ext, wk_ext,
              wv_ext, bv_ext, wp_ext, bp_ext, out_ext)
    nc.compile()
    return nc


def _body(tc, x_ext, sin_ext, cos_ext, wq_ext, bq_ext, wk_ext, wv_ext,
          bv_ext, wp_ext, bp_ext, out_ext):
    nc = tc.nc
    from concourse.masks import make_identity

    with contextlib.ExitStack() as ctx:
        persist = ctx.enter_context(tc.tile_pool(name="persist", bufs=1))

        xT = persist.tile([P, NKT, S], BF16)      # xT[p, t, s] = x[s, 128t+p]
        qT = persist.tile([P, NKT, S], BF16)      # roped q, [o, s]
        kT = persist.tile([P, NKT, S], BF16)
        wqT = persist.tile([P, NKT, H], BF16)     # wT[p, t, o] = W[o, 128t+p]
        wkT = persist.tile([P, NKT, H], BF16)
        wvT = persist.tile([P, NKT, H], BF16)
        wpT = persist.tile([P, NKT, H], BF16)
        vsb = persist.tile([P, NSTILE, NH, D + 1], BF16)  # v + ones column
        ctxT = persist.tile([P, NKT, S], BF16)    # normalized ctx^T
        # cos/sin tables, transposed, stacked in both 64-halves. One zero
        # pad column in front so rope ops span an EVEN 1370 elements from a
        # 4-byte-aligned base (lets the DVE run 2-elem/cycle bf16 mode).
        RW = 1 + NROT  # 1370
        cc2 = persist.tile([P, RW], BF16)
        ss2 = persist.tile([P, RW], BF16)
        bq_sb = persist.tile([P, NKT], F32)
        bv_row = persist.tile([1, H], BF16)
        bp_row = persist.tile([1, H], BF16)
        ones_row = persist.tile([1, P], BF16)
        ident = persist.tile([P, P], BF16)

        nc.vector.memset(ones_row, 1.0)
        nc.vector.memset(vsb[:, :, :, D:D + 1], 1.0)
        make_identity(nc, ident)

        # round-robin engines for staged loads (one DMA queue each)
        ld_engines = [nc.sync, nc.gpsimd, nc.scalar]
        ld_i = [0]

        # ---------------- setup: loads, casts, transposes ----------------
        with tc.tile_pool(name="stage", bufs=2) as stage, \
             tc.tile_pool(name="tpsum", bufs=4, space="PSUM") as tpsum:

            # biases
            nc.sync.dma_start(out=bq_sb,
                              in_=bq_ext.rearrange("(t p) -> p t", p=P))
            bs1 = stage.tile([1, H], F32, tag="bs1", bufs=1)
            nc.sync.dma_start(out=bs1, in_=bv_ext.rearrange("(a h) -> a h", a=1))
            nc.vector.tensor_copy(out=bv_row, in_=bs1)
            bs2 = stage.tile([1, H], F32, tag="bs2", bufs=1)
            nc.sync.dma_start(out=bs2, in_=bp_ext.rearrange("(a h) -> a h", a=1))
            nc.vector.tensor_copy(out=bp_row, in_=bs2)

            def load_cast(ext, rows, cols, tag):
                fs = stage.tile([P, cols], F32, tag=tag + "f", bufs=3)
                bs = stage.tile([P, cols], BF16, tag=tag + "b", bufs=3)
                ld_engines[ld_i[0] % 3].dma_start(out=fs[:rows], in_=ext)
                ld_i[0] += 1
                # cast on DVE (gpsimd casts measured 4x slower, gated setup)
                nc.vector.tensor_copy(out=bs[:rows], in_=fs[:rows])
                return bs

            ev_i = [0]

            def _evict(dst_ap, src_ap):
                # alternate DVE/ACT (gpsimd cannot read PSUM)
                k = ev_i[0] % 2
                ev_i[0] += 1
                if k == 0:
                    nc.vector.tensor_copy(out=dst_ap, in_=src_ap)
                else:
                    nc.scalar.copy(out=dst_ap, in_=src_ap)

            def pe_transpose_into(dst_ap, src_ap, rows, cols):
                # dst[cols, rows] = src[rows, cols].T  via PE + evict.
                # tile is padded to a full 2KB psum bank: matmul start=True
                # clears the whole bank, so sub-bank sharing races.
                tp = tpsum.tile([P, 1024], BF16, tag="tp")
                nc.tensor.transpose(tp[:cols, :rows], src_ap,
                                    ident[:rows, :rows])
                _evict(dst_ap, tp[:cols, :rows])

            # Wq / x / Wk / Wv: load, cast, PE-transpose (PE is idle in
            # setup; DMA-transpose was too slow to have WkT/WvT in time).
            # Emission order = DMA queue order: Wq and the first x tiles
            # first so q0-proj can start ASAP, Wk before the x tail so
            # k0-proj isn't gated on the last load.
            def w_tile(w_ext, wT, tg, r):
                wb = load_cast(w_ext[r * P:(r + 1) * P, :], P, H, tg)
                for t in range(NKT):
                    pe_transpose_into(wT[:, t, r * P:(r + 1) * P],
                                      wb[:, t * P:(t + 1) * P], P, P)

            def x_tile(st):
                s0, ssz = _stile(st)
                xb = load_cast(x_ext[s0:s0 + ssz, :], ssz, H, "x")
                for t in range(NKT):
                    pe_transpose_into(xT[:, t, s0:s0 + ssz],
                                      xb[:ssz, t * P:(t + 1) * P], ssz, P)

            # Wq + first x tiles first so q0-proj can start ASAP; Wk before
            # the x tail so k0-proj isn't gated on the last loads.
            for r in range(NKT):
                w_tile(wq_ext, wqT, "wq", r)
            for st in range(6):
                x_tile(st)
            for r in range(NKT):
                w_tile(wk_ext, wkT, "wk", r)
            for st in range(6, NSTILE):
                x_tile(st)
            for r in range(NKT):
                w_tile(wv_ext, wvT, "wv", r)

            # sin/cos: load, cast, PE-transpose into both 64-halves
            # (data starts at column 1; column 0 is the zero pad)
            nc.vector.memset(cc2[:, 0:1], 0.0)
            nc.vector.memset(ss2[:, 0:1], 0.0)
            n_rt = (NROT + P - 1) // P
            for src_ext, dstT in ((cos_ext, cc2), (sin_ext, ss2)):
                for i in range(n_rt):
                    r0 = i * P
                    rsz = min(P, NROT - r0)
                    cb = load_cast(src_ext[r0:r0 + rsz, :], rsz, D, "cs")
                    tp = tpsum.tile([P, 1024], BF16, tag="tp")
                    nc.tensor.transpose(tp[:D, :rsz], cb[:rsz, :D],
                                        ident[:rsz, :rsz])
                    for half in range(2):
                        nc.vector.tensor_copy(
                            out=dstT[D * half:D * half + D,
                                     1 + r0:1 + r0 + rsz],
                            in_=tp[:D, :rsz])
            # bake rotate_half sign into ss2: rows 0:32 and 64:96 negated
            for base in (0, 64):
                nc.vector.tensor_scalar_mul(ss2[base:base + 32, :],
                                            ss2[base:base + 32, :], -1.0)

            # Wp: load + cast + async DMA-transpose (needed only at the end)
            for r in range(NKT):
                wb = load_cast(wp_ext[r * P:(r + 1) * P, :], P, H, "wx")
                nc.scalar.dma_start_transpose(
                    out=wpT[:, :, r * P:(r + 1) * P], in_=wb)

        # ---------------- main pools ----------------
        qk_pool = ctx.enter_context(
            tc.tile_pool(name="qkps", bufs=1, space="PSUM"))
        pv_pool = ctx.enter_context(
            tc.tile_pool(name="pvps", bufs=2, space="PSUM"))
        es_pool = ctx.enter_context(tc.tile_pool(name="es", bufs=5))
        work = ctx.enter_context(tc.tile_pool(name="work", bufs=2))
        acc_pool = ctx.enter_context(tc.tile_pool(name="acc", bufs=2))
        norm_pool = ctx.enter_context(tc.tile_pool(name="norm", bufs=1))
        outst = ctx.enter_context(tc.tile_pool(name="outst", bufs=2))

        # ---------------- emission helpers ----------------
        def proj_T(wT, ot, dst, bias_col, un):
            """One i-chunk group of a transposed projection (q/k)."""
            i0, n = un
            ps = pv_pool.tile([P, BANK], F32, tag="pv",
                              name=f"pj_{ot}_{i0}")[:, :n]
            for kt in range(NKT):
                nc.tensor.matmul(ps, wT[:, kt, ot * P:(ot + 1) * P],
                                 xT[:, kt, i0:i0 + n],
                                 start=(kt == 0), stop=(kt == NKT - 1))
            # evict + bias on ACT (has headroom; DVE is the loaded engine)
            if bias_col is not None:
                nc.scalar.add(dst[:, i0:i0 + n], ps, bias_col)
            else:
                nc.scalar.copy(out=dst[:, i0:i0 + n], in_=ps)

        def rope(qb, ot, dstT):
            # rope math on DVE over an even-length window [PREFIX-1, S).
            # Column PREFIX-1 computes 0*qb + 0*rot = 0 and is then fixed
            # by the prefix copy (emitted last).
            rot = work.tile([P, RW], BF16, tag="rot", bufs=1,
                            name=f"rot_{ot}")
            for (dst0, src0) in ((0, 32), (32, 0), (64, 96), (96, 64)):
                nc.sync.dma_start(
                    out=rot[dst0:dst0 + 32, :],
                    in_=qb[src0:src0 + 32, PREFIX - 1:S])
            sl = slice(PREFIX - 1, S)
            nc.vector.tensor_mul(dstT[:, ot, sl], qb[:, sl], cc2)
            nc.vector.tensor_mul(rot, rot, ss2)
            nc.vector.tensor_add(dstT[:, ot, sl], dstT[:, ot, sl], rot)
            nc.vector.tensor_copy(out=dstT[:, ot, 0:PREFIX],
                                  in_=qb[:, 0:PREFIX])

        def make_proj_units(which, wT, ot, dstT, bias_col):
            # allocate qb at emission of the first chunk; rope reads it last
            box = {}
            units = []
            for ci, un in enumerate(ICH3):
                def chunk(u=un, first=(ci == 0)):
                    if first:
                        box["qb"] = work.tile([P, S], BF16,
                                              tag=f"qb{which}", bufs=1,
                                              name=f"qb_{which}_{ot}")
                    proj_T(wT, ot, box["qb"], bias_col, u)
                units.append(((f"pc_{which}", ot, un[0]), chunk))
            units.append(((which, ot),
                          lambda: rope(box["qb"], ot, dstT)))
            return units

        def v_chunk(st, ch):
            o0, n = VCH[ch]
            s0, ssz = _stile(st)
            ps = pv_pool.tile([P, BANK], F32, tag="pv",
                              name=f"vp_{st}_{ch}")
            for kt in range(NKT):
                nc.tensor.matmul(ps[:ssz, :n], xT[:, kt, s0:s0 + ssz],
                                 wvT[:, kt, o0:o0 + n],
                                 start=(kt == 0), stop=False)
            nc.tensor.matmul(ps[:ssz, :n], ones_row[:, :ssz],
                             bv_row[:, o0:o0 + n], start=False, stop=True)
            nc.scalar.copy(
                out=vsb[:ssz, st, o0 // D:(o0 + n) // D, 0:D],
                in_=ps[:ssz, :n].rearrange("p (h d) -> p h d", d=D))

        # filler queue: ordered (key, emit_fn), popped with deadline checks
        fillers = []
        emitted = set()

        def drain(n):
            for _ in range(min(n, len(fillers))):
                k, fn = fillers.pop(0)
                fn()
                emitted.add(k)

        def drain_until(key):
            if key in emitted:
                return
            assert any(k == key for k, _ in fillers), f"missing filler {key}"
            while fillers:
                k, fn = fillers.pop(0)
                fn()
                emitted.add(k)
                if k == key:
                    return

        # ---------------- attention emitters ----------------
        es_tiles = {}
        acc_tiles = {}

        def emit_qk_j(pt, jt):
            j0, jsz = _stile(jt)
            for hh in range(2):
                hb = 64 * hh
                qk = qk_pool.tile([P, 3 * BANK], F32, tag=f"qk{hh}",
                                  bufs=1, name=f"qk_{pt}_{jt}_{hh}")
                for (i0, n) in ICH3:
                    nc.tensor.matmul(
                        qk[:jsz, i0:i0 + n],
                        kT[hb:hb + D, pt, j0:j0 + jsz],
                        qT[hb:hb + D, pt, i0:i0 + n],
                        start=True, stop=True)
                es = es_pool.tile([P, S], BF16, tag=f"es{hh}", bufs=5,
                                  name=f"es_{pt}_{jt}_{hh}")
                nc.scalar.activation(out=es[:jsz, :], in_=qk[:jsz, :S],
                                     func=EXP, scale=float(D) ** -0.5)
                es_tiles[(jt, hh)] = es

        def pv_piece(ppt, pw, hh, i0, n):
            """One (head, i-chunk) PV psum group over window pw's j-tiles,
            DVE-accumulated into the pair's SBUF accumulator."""
            h = 2 * ppt + hh
            if pw == 0 and i0 == 0:
                acc_tiles[hh] = acc_pool.tile(
                    [D + 1, S], F32, tag=f"acc{hh}", bufs=2,
                    name=f"acc_{ppt}_{hh}")
            a = acc_tiles[hh]
            pjs = JWIN[pw]
            ps = pv_pool.tile([P, BANK], F32, tag="pv",
                              name=f"pv_{ppt}_{pw}_{hh}_{i0}")
            for ji, jt in enumerate(pjs):
                j0, jsz = _stile(jt)
                nc.tensor.matmul(
                    ps[:D + 1, :n], vsb[:jsz, jt, h, :],
                    es_tiles[(jt, hh)][:jsz, i0:i0 + n],
                    start=(ji == 0), stop=(ji == len(pjs) - 1))
            # first window initializes the accumulator via ACT (idle at
            # pair starts); later windows accumulate on DVE (ACT can't add
            # two tensors, and gpsimd cannot read PSUM)
            if pw == 0:
                nc.scalar.copy(out=a[:, i0:i0 + n], in_=ps[:D + 1, :n])
            else:
                nc.vector.scalar_tensor_tensor(
                    out=a[:, i0:i0 + n], in0=ps[:D + 1, :n],
                    scalar=0.0, in1=a[:, i0:i0 + n],
                    op0=ALU.add, op1=ALU.add)

        def emit_norm(pt):
            for hh in range(2):
                a = acc_tiles[hh]
                # partition_broadcast only reads absolute partition 0 on HW:
                # move the denominator row there first (cross-base DVE copy).
                row0 = norm_pool.tile([1, S], F32, tag="row0", bufs=1,
                                      name=f"row0_{pt}_{hh}")
                nc.vector.tensor_copy(out=row0, in_=a[D:D + 1, :])
                br = norm_pool.tile([D, S], F32, tag="br", bufs=1,
                                    name=f"br_{pt}_{hh}")
                nc.gpsimd.partition_broadcast(out_ap=br[:, :], in_ap=row0,
                                              channels=D)
                nc.vector.reciprocal_approx_fast(out=br[:, :], in_=br[:, :])
                nc.vector.tensor_mul(ctxT[64 * hh:64 * hh + D, pt, :],
                                     a[0:D, :], br[:, :])

        # ---------------- pre-roll: q0, k0, v chunk0 st0..5 ----------------
        for key, fn in make_proj_units("q", wqT, 0, qT, bq_sb[:, 0:1]):
            fn()
            emitted.add(key)
        for key, fn in make_proj_units("k", wkT, 0, kT, None):
            fn()
            emitted.add(key)
        for st in range(6):
            v_chunk(st, 0)
            emitted.add(("v", 0, st))

        # ---------------- fill the filler queue ----------------
        for st in range(6, NSTILE):
            fillers.append((("v", 0, st), lambda s=st: v_chunk(s, 0)))
        for ot in range(1, NKT):
            for u in make_proj_units("q", wqT, ot, qT, bq_sb[:, ot:ot + 1]):
                fillers.append(u)
            for u in make_proj_units("k", wkT, ot, kT, None):
                fillers.append(u)
            if ot == 1:
                for st in range(NSTILE):
                    fillers.append((("v", 1, st), lambda s=st: v_chunk(s, 1)))

        # ---------------- attention pipeline ----------------
        # software pipeline: while window w's QK/exp stream, the PREVIOUS
        # window's PV pieces are interleaved BETWEEN the QK j-steps so the
        # in-order PE queue always has ready work ahead of an exp-wait.
        def window_pieces(prev):
            if prev is None:
                return []
            ppt, pw = prev
            vch = 0 if ppt < 4 else 1
            for jt in JWIN[pw]:
                drain_until(("v", vch, jt))
            return [(ppt, pw, hh, i0, n)
                    for hh in range(2) for (i0, n) in ICH3]

        seq = [(pt, w) for pt in range(NPAIR) for w in range(len(JWIN))]
        prev = None
        for (pt, w) in seq:
            if w == 0 and pt > 0:
                drain_until(("k", pt))  # q/k proj + rope for this pair done
            js = JWIN[w]
            pieces = window_pieces(prev)
            k = 0
            for ji, jt in enumerate(js):
                emit_qk_j(pt, jt)
                want = (len(pieces) * (ji + 1)) // len(js)
                while k < want:
                    pv_piece(*pieces[k])
                    k += 1
                drain(1)
            while k < len(pieces):
                pv_piece(*pieces[k])
                k += 1
            if prev is not None and prev[1] == len(JWIN) - 1:
                emit_norm(prev[0])
            prev = (pt, w)
        for piece in window_pieces(prev):
            pv_piece(*piece)
        emit_norm(prev[0])
        drain(len(fillers))

        # ---------------- output projection ----------------
        for st in range(NSTILE):
            s0, ssz = _stile(st)
            ot_t = outst.tile([P, H], F32, tag="ost", name=f"ost_{st}")
            for (o0, n) in VCH:
                ps = pv_pool.tile([P, BANK], F32, tag="pv",
                                  name=f"ops_{st}_{o0}")
                for kt in range(NKT):
                    nc.tensor.matmul(ps[:ssz, :n],
                                     ctxT[:, kt, s0:s0 + ssz],
                                     wpT[:, kt, o0:o0 + n],
                                     start=(kt == 0), stop=False)
                nc.tensor.matmul(ps[:ssz, :n], ones_row[:, :ssz],
                                 bp_row[:, o0:o0 + n],
                                 start=False, stop=True)
                # evict on ACT: exp work is finished by out-proj time
                nc.scalar.copy(out=ot_t[:ssz, o0:o0 + n],
                               in_=ps[:ssz, :n])
            nc.sync.dma_start(out=out_ext[s0:s0 + ssz, :], in_=ot_t[:ssz])


_NC_CACHE = None


def get_nc():
    global _NC_CACHE
    if _NC_CACHE is None:
        nc = bacc.Bacc(None, target_bir_lowering=False, debug=False)
        _NC_CACHE = build_kernel(nc)
    return _NC_CACHE


def kernel(**inputs):
    from concourse.bass_utils import run_bass_kernel_spmd

    nc = get_nc()
    names = ["hidden_states", "sin", "cos", "Wq", "bq", "Wk", "Wv", "bv",
             "Wp", "bp"]
    arrs = {k: np.ascontiguousarray(np.asarray(inputs[k], dtype=np.float32))
            for k in names}
    in_maps = []
    for b in range(B):
        m = {k: arrs[k] for k in names if k != "hidden_states"}
        m["hidden_states"] = np.ascontiguousarray(arrs["hidden_states"][b])
        in_maps.append(m)
    res = run_bass_kernel_spmd(nc, in_maps, core_ids=list(range(B)))
    out = np.stack([res.results[b]["out"] for b in range(B)], axis=0)
    return out.astype(np.float32)


if __name__ == "__main__":
    nc = get_nc()
    print("built ok")
